# revision 14
# baseline (speedup 1.0000x reference)
"""Trainium2 Bass kernel: causal multi-head attention with RoPE.

Problem: B=2, S=2048, D=1024, H=16 heads, hd=64, fp32.
Sharding: 4-way head-tensor-parallel x 2-way batch-data-parallel over 8 cores.
Each core handles one batch element and 4 heads (256 of the 1024 model dims),
computes its partial contribution to the output projection, and the host sums
the 4 partials per batch element.

Per-core pipeline (matmuls in fp32r ~ tf32-grade precision):
  - x arrives split as bf16 hi/lo pairs; the device transposes both with the
    DMA xbar (2-byte-only path) and reconstructs xT = hi + lo on DVE in f32r.
  - RoPE via double projection: host supplies row-permuted/negated weight
    copies W1/W2 so q_rot = (x@W1.T).T * cos + (x@W2.T).T * sin needs no
    cross-partition shuffles on device.
  - scores^T = k_rot^T.T @ q_rot^T per (head, q-chunk, k-block-pair) with 2x
    row-tiling (K=64) packing head pairs on the PE array; causal block skip.
  - exp on ACT over two k-blocks at a time (scale=1/8 fused); the two
    diagonal block-pairs are masked multiplicatively with precomputed pair
    masks (on GpSimd to keep DVE free).
  - P^T @ v via matmul with a ones-column appended to v (M=65) so the
    softmax denominator accumulates for free in PSUM row 64.
  - normalization deferred past the attention loop: 1/denom via ln/exp on
    ACT (batched so the ACT table set only switches twice), broadcast
    across partitions with a K=1 matmul, normalize O^T on DVE.
  - y_partial = O_norm @ Wo_slice.T accumulated over the 4 heads (K=64).
"""
import numpy as np
import ml_dtypes
from contextlib import ExitStack

import concourse.bass as bass
import concourse.tile as tile
from concourse import bacc, mybir
from concourse.bass_utils import run_bass_kernel_spmd

F32 = mybir.dt.float32
F32R = mybir.dt.float32r
BF16 = mybir.dt.bfloat16

B, S, D, H, HD = 2, 2048, 1024, 16, 64
NCORES = 8
TPG = 4            # head-TP degree (groups of 4 heads)
LH = H // TPG      # 4 local heads per core
LD = LH * HD       # 256 local dims
ROPE_BASE = 10000.0
QC = 512           # q chunk (matmul moving dim)
NQC = S // QC      # 4
NST = S // 128     # 16 s tiles
NDT = D // 128     # 8 d tiles

Exp = mybir.ActivationFunctionType.Exp
Log = mybir.ActivationFunctionType.Ln

_NC_CACHE = None


def _build():
    nc = bacc.Bacc("TRN2", target_bir_lowering=False, debug=False,
                   enable_asserts=True, num_devices=NCORES)

    xhi = nc.dram_tensor("xhi", [NDT, S, 128], BF16, kind="ExternalInput").ap()
    xlo = nc.dram_tensor("xlo", [NDT, S, 128], BF16, kind="ExternalInput").ap()
    w1qt = nc.dram_tensor("w1qt", [128, 2048], F32, kind="ExternalInput").ap()
    w2qt = nc.dram_tensor("w2qt", [128, 2048], F32, kind="ExternalInput").ap()
    w1kt = nc.dram_tensor("w1kt", [128, 2048], F32, kind="ExternalInput").ap()
    w2kt = nc.dram_tensor("w2kt", [128, 2048], F32, kind="ExternalInput").ap()
    wvt = nc.dram_tensor("wvt", [128, 2048], F32, kind="ExternalInput").ap()
    wot = nc.dram_tensor("wot", [LH, HD, D], F32, kind="ExternalInput").ap()
    cos2 = nc.dram_tensor("cos2", [128, S], F32, kind="ExternalInput").ap()
    sin2 = nc.dram_tensor("sin2", [128, S], F32, kind="ExternalInput").ap()
    pairmask = nc.dram_tensor("pairmask", [2, 128, 1024], F32,
                              kind="ExternalInput").ap()
    y = nc.dram_tensor("y", [S, D], F32, kind="ExternalOutput").ap()

    with tile.TileContext(nc) as tc, ExitStack() as octx:
        # ---- persistent pools ----
        pers = octx.enter_context(tc.tile_pool(name="pers", bufs=1))
        qkp = octx.enter_context(tc.tile_pool(name="qkp", bufs=1))
        vp = octx.enter_context(tc.tile_pool(name="vp", bufs=1))

        ones_f = pers.tile([128, 512], F32, tag="ones_f")
        nc.vector.memset(ones_f[:], 1.0)
        ones64 = pers.tile([1, 64], F32R, tag="ones64")
        nc.vector.tensor_copy(ones64[:], ones_f[0:1, 0:64])

        # q/k rotated, per head-pair tile: rows = [hA:(y1 32|y2 32) | hB:...]
        qrot = [qkp.tile([128, S], F32R, tag=f"qrot{j}", name=f"qrot{j}")
                for j in range(2)]
        krot = [qkp.tile([128, S], F32R, tag=f"krot{j}", name=f"krot{j}")
                for j in range(2)]
        # v natural with per-head ones column: cols lh*65..lh*65+64 = v head
        # lh, col lh*65+64 = 1.0
        vsb = [vp.tile([128, 260], F32R, tag=f"v{st}", name=f"v{st}")
               for st in range(NST)]

        with ExitStack() as s1:
            s1p = s1.enter_context(tc.tile_pool(name="s1p", bufs=1))
            xtp = s1.enter_context(tc.tile_pool(name="xtp", bufs=1))
            ppps = s1.enter_context(tc.tile_pool(name="ppps", bufs=4, space="PSUM"))
            pvps = s1.enter_context(tc.tile_pool(name="pvps", bufs=2, space="PSUM"))

            xt = [xtp.tile([128, S], F32R, tag=f"xt{dt}", name=f"xt{dt}")
                  for dt in range(NDT)]

            # ---- phase T first: transpose x via DMA xbar + hi/lo recombine.
            # Emitted before any other DMA so the xbar transposes (the
            # critical path into the projections) aren't queued behind the
            # weight loads.
            for dt in range(NDT):
                ht = s1p.tile([128, S], BF16, tag="hilo", bufs=4, name="ht")
                nc.sync.dma_start_transpose(ht[:], xhi[dt])
                lt = s1p.tile([128, S], BF16, tag="hilo", bufs=4, name="lt")
                nc.sync.dma_start_transpose(lt[:], xlo[dt])
                nc.vector.tensor_add(xt[dt][:], ht[:], lt[:])

            cos_sb = s1p.tile([128, S], F32, tag="cos")
            nc.sync.dma_start(cos_sb[:], cos2)
            sin_sb = s1p.tile([128, S], F32, tag="sin")
            nc.sync.dma_start(sin_sb[:], sin2)
            wq1 = s1p.tile([128, 2048], F32R, tag="wq1")
            nc.gpsimd.dma_start(wq1[:], w1qt)
            wq2 = s1p.tile([128, 2048], F32R, tag="wq2")
            nc.gpsimd.dma_start(wq2[:], w2qt)
            wk1 = s1p.tile([128, 2048], F32R, tag="wk1")
            nc.gpsimd.dma_start(wk1[:], w1kt)
            wk2 = s1p.tile([128, 2048], F32R, tag="wk2")
            nc.gpsimd.dma_start(wk2[:], w2kt)
            wv = s1p.tile([128, 2048], F32R, tag="wv")
            nc.gpsimd.dma_start(wv[:], wvt)

            # ---- phase P: q/k projections + RoPE (both head-pair tiles) ----
            for (w1, w2, rot) in ((wq1, wq2, qrot), (wk1, wk2, krot)):
                for jt in range(2):
                    for sc in range(NQC):
                        p1 = ppps.tile([128, QC], F32, tag="pp", name="p1")
                        for dt in range(NDT):
                            nc.tensor.matmul(
                                p1[:],
                                w1[:, dt * 256 + jt * 128: dt * 256 + jt * 128 + 128],
                                xt[dt][:, sc * QC:(sc + 1) * QC],
                                start=(dt == 0), stop=(dt == NDT - 1))
                        p2 = ppps.tile([128, QC], F32, tag="pp", name="p2")
                        for dt in range(NDT):
                            nc.tensor.matmul(
                                p2[:],
                                w2[:, dt * 256 + jt * 128: dt * 256 + jt * 128 + 128],
                                xt[dt][:, sc * QC:(sc + 1) * QC],
                                start=(dt == 0), stop=(dt == NDT - 1))
                        t1 = s1p.tile([128, QC], F32, tag="rt", bufs=4, name="t1")
                        nc.vector.tensor_mul(t1[:], p1[:],
                                             cos_sb[:, sc * QC:(sc + 1) * QC])
                        t2 = s1p.tile([128, QC], F32, tag="rt", bufs=4, name="t2")
                        nc.vector.tensor_mul(t2[:], p2[:],
                                             sin_sb[:, sc * QC:(sc + 1) * QC])
                        nc.vector.tensor_add(rot[jt][:, sc * QC:(sc + 1) * QC],
                                             t1[:], t2[:])

            # ---- phase V: v projection ----
            for st in range(NST):
                # ones cols at 64,129,194,259
                vdst = vsb[st].rearrange("p (h c) -> p h c", c=65)[:, :, 64:65]
                nc.vector.tensor_copy(vdst, ones_f[:, 0:4].rearrange(
                    "p (h c) -> p h c", c=1))
                pv = pvps.tile([128, 256], F32, tag="pv", name="pv")
                for dt in range(NDT):
                    nc.tensor.matmul(pv[:],
                                     xt[dt][:, st * 128:(st + 1) * 128],
                                     wv[:, dt * 256:(dt + 1) * 256],
                                     start=(dt == 0), stop=(dt == NDT - 1))
                # strided copy into per-head 65-col groups
                dst = vsb[st].rearrange("p (h c) -> p h c", c=65)[:, :, 0:64]
                src = pv.rearrange("p (h c) -> p h c", c=64)
                nc.scalar.copy(dst, src)

        # ---- attention-persistent tiles ----
        ap = octx.enter_context(tc.tile_pool(name="ap", bufs=1))
        masks = []
        for j in range(2):
            m = ap.tile([128, 1024], F32R, tag=f"mask{j}", name=f"m{j}")
            nc.gpsimd.dma_start(m[:], pairmask[j])
            masks.append(m)
        wo_sb = []
        for lh in range(LH):
            w = ap.tile([64, D], F32R, tag=f"wo{lh}", name=f"wo{lh}")
            nc.gpsimd.dma_start(w[:], wot[lh])
            wo_sb.append(w)
        # O^T unnormalized + denominator row, per head: rows 0:64 = O^T,
        # row 64 = sum(exp) denominator
        otu = [ap.tile([65, S], F32, tag=f"otu{lh}", name=f"otu{lh}")
               for lh in range(LH)]
        # 1/denominator per head, [1, S]
        rsb = [ap.tile([1, S], F32R, tag=f"rsb{lh}", name=f"rsb{lh}")
               for lh in range(LH)]

        # ---- phase A: attention ----
        with ExitStack() as s2:
            s2p = s2.enter_context(tc.tile_pool(name="s2p", bufs=1))
            pss = s2.enter_context(tc.tile_pool(name="pss", bufs=3, space="PSUM"))
            pso = s2.enter_context(tc.tile_pool(name="pso", bufs=2, space="PSUM"))

            for hp in range(2):          # head pair = (2hp, 2hp+1)
                for qc in range(NQC):
                    npair = 2 * qc + 2
                    po = [pso.tile([128, QC], F32, tag="po", name="po")
                          for _ in range(2)]

                    def emit_pv(kp, pts, hp=hp, qc=qc, po=po):
                        kb0 = 2 * kp
                        for z in range(2):
                            lh = 2 * hp + z
                            for e in range(2):
                                kb = kb0 + e
                                nc.tensor.matmul(
                                    po[z][0:65, :],
                                    vsb[kb][:, lh * 65:lh * 65 + 65],
                                    pts[z][:, e * QC:(e + 1) * QC],
                                    start=(kb == 0),
                                    stop=(kb == 4 * qc + 3))

                    prev = None
                    for kp in range(npair):
                        kb0 = 2 * kp
                        pts = []
                        for z in range(2):   # z=0: rows 0:64, z=1: 64:128
                            r0 = 64 * z
                            ps_ = pss.tile([128, 1024], F32, tag="ps", name="ps_")
                            for e in range(2):
                                kb = kb0 + e
                                nc.tensor.matmul(
                                    ps_[:, e * QC:(e + 1) * QC],
                                    krot[hp][r0:r0 + 64, kb * 128:(kb + 1) * 128],
                                    qrot[hp][r0:r0 + 64, qc * QC:(qc + 1) * QC],
                                    start=True, stop=True, tile_position=(r0, 0))
                            pt = s2p.tile([128, 1024], F32R, tag="pt", bufs=6,
                                          name="pt")
                            nc.scalar.activation(pt[:], ps_[:], Exp, scale=0.125)
                            if kp >= npair - 2:  # diagonal block pair
                                j = 0 if kp == npair - 2 else 1
                                eng = nc.vector if z == 0 else nc.gpsimd
                                eng.tensor_mul(pt[:], pt[:], masks[j][:])
                            pts.append(pt)
                        # software-pipeline: PV for the previous round, so the
                        # PE never waits on the exp/mask of the current one.
                        if prev is not None:
                            emit_pv(kp - 1, prev)
                        prev = pts
                    emit_pv(npair - 1, prev)
                    for z in range(2):
                        lh = 2 * hp + z
                        nc.vector.tensor_copy(
                            otu[lh][:, qc * QC:(qc + 1) * QC], po[z][0:65, :])
                # denominator reciprocals for this head pair (ln+exp batches;
                # runs on ACT during the next head pair's attention)
                for z in range(2):
                    lh = 2 * hp + z
                    lnd = s2p.tile([1, S], F32, tag="lnd", bufs=2, name="lnd")
                    nc.scalar.activation(lnd[:], otu[lh][64:65, :], Log)
                    nc.scalar.activation(rsb[lh][:], lnd[:], Exp, scale=-1.0)

        # ---- phases N+Y interleaved: normalize + output projection ----
        with ExitStack() as s3:
            s3p = s3.enter_context(tc.tile_pool(name="s3p", bufs=1))
            otnp = s3.enter_context(tc.tile_pool(name="otnp", bufs=1))
            psb = s3.enter_context(tc.tile_pool(name="psb", bufs=2, space="PSUM"))
            psy = s3.enter_context(tc.tile_pool(name="psy", bufs=3, space="PSUM"))

            otn = [otnp.tile([64, S], F32R, tag=f"otn{lh}", name=f"otn{lh}")
                   for lh in range(LH)]
            for qc in range(NQC):
                for lh in range(LH):
                    pb = psb.tile([128, QC], F32, tag="pb", name="pb")
                    nc.tensor.matmul(pb[0:64, :], ones64[:],
                                     rsb[lh][0:1, qc * QC:(qc + 1) * QC],
                                     start=True, stop=True)
                    nc.vector.tensor_mul(
                        otn[lh][:, qc * QC:(qc + 1) * QC],
                        pb[0:64, :], otu[lh][0:64, qc * QC:(qc + 1) * QC])
                for st in range(4 * qc, 4 * qc + 4):
                    ysb = s3p.tile([128, D], F32, tag="ysb", bufs=3, name="ysb")
                    for mc in range(2):
                        py = psy.tile([128, 512], F32, tag="py", name="py")
                        for lh in range(LH):
                            nc.tensor.matmul(
                                py[:],
                                otn[lh][:, st * 128:(st + 1) * 128],
                                wo_sb[lh][:, mc * 512:(mc + 1) * 512],
                                start=(lh == 0), stop=(lh == LH - 1))
                        dst = ysb[:, mc * 512:(mc + 1) * 512]
                        if mc == 0:
                            nc.scalar.copy(dst, py[:])
                        else:
                            nc.vector.tensor_copy(dst, py[:])
                    nc.sync.dma_start(y[st * 128:(st + 1) * 128, :], ysb[:])

    nc.compile()
    return nc


def _get_nc():
    global _NC_CACHE
    if _NC_CACHE is None:
        _NC_CACHE = _build()
    return _NC_CACHE


def _host_prep(x, Wq, Wk, Wv, Wo):
    """Build the 8 per-core input maps."""
    x = np.asarray(x, dtype=np.float32)
    Wq, Wk, Wv, Wo = (np.asarray(w, dtype=np.float32) for w in (Wq, Wk, Wv, Wo))

    def tile128(wt):  # [1024, 256] -> [128, 2048] with d-tiles along free dim
        return np.ascontiguousarray(
            wt.reshape(NDT, 128, LD).transpose(1, 0, 2).reshape(128, NDT * LD))

    def perm_pair(W, g):
        blocks1, blocks2 = [], []
        for lh in range(LH):
            gh = g * LH + lh
            O = W[gh * HD:(gh + 1) * HD]          # [64, 1024]
            ev, od = O[0::2], O[1::2]
            blocks1.append(np.concatenate([ev, od], axis=0))
            blocks2.append(np.concatenate([-od, ev], axis=0))
        W1 = np.concatenate(blocks1, axis=0)      # [256, 1024]
        W2 = np.concatenate(blocks2, axis=0)
        return tile128(W1.T), tile128(W2.T)

    t = np.arange(32, dtype=np.float64)
    theta = 1.0 / (ROPE_BASE ** (2.0 * t / HD))
    ang = np.arange(S, dtype=np.float64)[:, None] * theta[None, :]  # [S, 32]
    c32 = np.cos(ang).T.astype(np.float32)        # [32, S]
    s32 = np.sin(ang).T.astype(np.float32)
    cos2 = np.ascontiguousarray(np.tile(c32, (4, 1)))   # [128, S]
    sin2 = np.ascontiguousarray(np.tile(s32, (4, 1)))

    # pair masks for the two diagonal block-pairs of each (z, qc):
    # mask[j][p, u]: j=0 covers relative blocks (0, 1), j=1 covers (2, 3).
    p = np.arange(128)[:, None]
    u = np.arange(1024)[None, :]
    pm = np.zeros((2, 128, 1024), dtype=np.float32)
    for j in range(2):
        off = 256 * j
        pm[j] = np.where(u < 512, p <= u - off, p <= u - 640 - off)
    pairmask = np.ascontiguousarray(pm)

    bf = ml_dtypes.bfloat16
    per_b = []
    for b in range(B):
        xhi = x[b].astype(bf)
        xlo = (x[b] - xhi.astype(np.float32)).astype(bf)
        # d-tile-major [NDT, S, 128] so each xbar transpose reads one
        # fully-contiguous 512 KB block
        xhi = xhi.reshape(S, NDT, 128).transpose(1, 0, 2)
        xlo = xlo.reshape(S, NDT, 128).transpose(1, 0, 2)
        per_b.append((np.ascontiguousarray(xhi), np.ascontiguousarray(xlo)))

    per_g = []
    for g in range(TPG):
        w1q, w2q = perm_pair(Wq, g)
        w1k, w2k = perm_pair(Wk, g)
        wvt_ = tile128(Wv[g * LD:(g + 1) * LD].T)
        wot_ = np.ascontiguousarray(
            Wo[:, g * LD:(g + 1) * LD].T.reshape(LH, HD, D))
        per_g.append((w1q, w2q, w1k, w2k, wvt_, wot_))

    in_maps = []
    for c in range(NCORES):
        b, g = divmod(c, TPG)
        w1q, w2q, w1k, w2k, wvt_, wot_ = per_g[g]
        in_maps.append({
            "xhi": per_b[b][0], "xlo": per_b[b][1],
            "w1qt": w1q, "w2qt": w2q, "w1kt": w1k, "w2kt": w2k,
            "wvt": wvt_, "wot": wot_,
            "cos2": cos2, "sin2": sin2,
            "pairmask": pairmask,
        })
    return in_maps


def run(inputs, trace=False):
    """Run on all 8 cores; returns (y_full, BassKernelResults)."""
    x = inputs["x"]
    in_maps = _host_prep(x, inputs["Wq"], inputs["Wk"], inputs["Wv"],
                         inputs["Wo"])
    nc = _get_nc()
    kw = {}
    if trace:
        kw = dict(trace=True, trace_cores=[0])
    res = run_bass_kernel_spmd(nc, in_maps, core_ids=list(range(NCORES)), **kw)
    y = np.zeros((B, S, D), dtype=np.float32)
    for c in range(NCORES):
        y[c // TPG] += res.results[c]["y"]
    return y, res


def kernel(x, Wq, Wk, Wv, Wo, n_heads):
    assert int(n_heads) == H
    y, _ = run({"x": x, "Wq": Wq, "Wk": Wk, "Wv": Wv, "Wo": Wo})
    return y
